# revision 7
# baseline (speedup 1.0000x reference)
"""Bidirectional complex-diagonal LRU (Linear Recurrent Unit) on 8 Trainium2 cores.

Math: lam = exp(-exp(nu_log) + i*exp(theta_log)) per channel n (N=512).
  Bu = einsum('blh,hn->bnl', u, B0 + iB1), masked to length.
  Forward scan over channels [0,256), backward (time-reversed) over [256,512).
  y = x.real @ C0 - x.imag @ C1, zeroed past each sequence length.

Device strategy (data-parallel, one batch per core):
  - Rotation trick: x_t = e^{i*th*t} * w_t turns the complex recurrence
    x_t = lam x_{t-1} + Bu_t into TWO real recurrences w_t = r w_{t-1} + v_t
    (r = |lam|), each a native DVE tensor_tensor_scan along the free dim.
  - Twiddle tables cos/sin(th*j) built on host in fp64 (exact phases), fp16 on
    device. Per-core masking (zero columns past the sequence length) is folded
    into the tables, so masking costs nothing on device.
  - Backward channels run on the reversed time axis; reversal happens inside
    the PSUM-evacuation copy (negative-stride AP) and the untwiddle writes.
  - All matmuls fp16 (full PE rate), accumulation in fp32 PSUM. Scans run
    in place over the v buffer; x overwrites v (fwd) / spent cos+sin table
    slices (bwd), so SBUF holds everything with no extra big buffers.
  - Elementwise combine ops are split DVE/GPSIMD to balance engine load.

Self-contained: hardcodes B=8, L=4096, H=N=512, 8 cores.
"""

import numpy as np
from contextlib import ExitStack

import concourse.bass as bass
import concourse.bacc as bacc
import concourse.mybir as mybir
import concourse.tile as tile

P = 128
L = 4096
H = 512
N = 512
BSZ = 8
SEG = 512                # Bu matmul / evac granularity (one PSUM bank)
NSEG = L // SEG          # 8
SLAB = 1024              # scan + untwiddle granularity
NSLAB = L // SLAB        # 4
TSLAB = 2048             # twiddle-in granularity
NTSLAB = L // TSLAB      # 2
KH = H // P              # 4 contraction chunks for Bu
NCH = 2 * N // P         # 8 real-channel chunks (re 0..3, im 4..7)
CCH = N // P             # 4 complex-channel chunks (0,1 fwd; 2,3 bwd)
NT = L // P              # 32 time blocks for the output matmul

F16 = mybir.dt.float16
F32 = mybir.dt.float32
MULT = mybir.AluOpType.mult
ADD = mybir.AluOpType.add

C_ORDER = [0, 2, 1, 3]

_CACHED = None


def _is_fwd_chunk(nch: int) -> bool:
    return (nch % 4) < 2


def build_nc():
    nc = bacc.Bacc("TRN2", target_bir_lowering=False, debug=False)
    uT = nc.declare_dram_parameter("uT", [H, L], F16, isOutput=False)
    cosT = nc.declare_dram_parameter("cosT", [N, L], F16, isOutput=False)
    sinT = nc.declare_dram_parameter("sinT", [N, L], F16, isOutput=False)
    rdec = nc.declare_dram_parameter("rdec", [P, CCH], F32, isOutput=False)
    Bcat = nc.declare_dram_parameter("Bcat", [H, 2 * N], F16, isOutput=False)
    Ccat = nc.declare_dram_parameter("Ccat", [2 * N, H], F16, isOutput=False)
    y = nc.declare_dram_parameter("y", [L, H], F32, isOutput=True)

    with tile.TileContext(nc) as tc, ExitStack() as ctx:
        const = ctx.enter_context(tc.tile_pool(name="const", bufs=1))
        big = ctx.enter_context(tc.tile_pool(name="big", bufs=1))
        upool = ctx.enter_context(tc.tile_pool(name="upool", bufs=1))
        pscr = ctx.enter_context(tc.tile_pool(name="pscr", bufs=6))
        qscr = ctx.enter_context(tc.tile_pool(name="qscr", bufs=8))
        ysb = ctx.enter_context(tc.tile_pool(name="ysb", bufs=3))
        crp = ctx.enter_context(tc.tile_pool(name="crp", bufs=16))
        bup = ctx.enter_context(tc.tile_pool(name="bup", bufs=6, space="PSUM"))
        yp = ctx.enter_context(tc.tile_pool(name="yp", bufs=2, space="PSUM"))

        # uT streamed in halves: cols [0:2048] then [2048:4096]
        u_t = [upool.tile([P, TSLAB], F16, tag=f"uT{k}", name=f"uT{k}")
               for k in range(KH)]
        cosb = [big.tile([P, L], F16, tag=f"cos{c}", name=f"cos{c}") for c in range(CCH)]
        sinb = [big.tile([P, L], F16, tag=f"sin{c}", name=f"sin{c}") for c in range(CCH)]
        v = [big.tile([P, L], F16, tag=f"v{j}", name=f"v{j}") for j in range(NCH)]
        bmat = [const.tile([P, 2 * N], F16, tag=f"B{k}", name=f"Bm{k}") for k in range(KH)]
        cmat = [const.tile([P, H], F16, tag=f"C{k}", name=f"Cm{k}") for k in range(NCH)]
        rdec_t = const.tile([P, CCH], F32, tag="rdec", name="rdec_t")

        # ---- constant DMAs ----
        nc.sync.dma_start(rdec_t[:], rdec[:])
        for k in range(KH):
            nc.sync.dma_start(bmat[k][:], Bcat[k * P:(k + 1) * P, :])
        for k in range(NCH):
            nc.sync.dma_start(cmat[k][:], Ccat[k * P:(k + 1) * P, :])
        for c in range(CCH):
            nc.sync.dma_start(cosb[c][:], cosT[c * P:(c + 1) * P, :])
            nc.sync.dma_start(sinb[c][:], sinT[c * P:(c + 1) * P, :])

        # ---- Phase A: Bu matmuls, evacuate into v slots (scan-time order) ----
        # uhalf 0 covers tsegs {0..3} (first-needed by fwd chunks), uhalf 1
        # covers tsegs {4..7} (first-needed by bwd chunks, reversed).  All
        # half-0 groups run first (uT tiles are reloaded for half 1); within
        # each half, chunk-pair priority interleave.
        g_half0 = [0, 4, 2, 6, 1, 5, 3, 7]
        g_half1 = [2, 6, 0, 4, 3, 7, 1, 5]

        def do_group(nch, h):
            tsegs = [0, 1, 2, 3] if h == 0 else [7, 6, 5, 4]
            psums = {}
            for k in range(KH):
                for ts in tsegs:
                    if k == 0:
                        psums[ts] = bup.tile([P, SEG], F32, name=f"bups{ts}",
                                             tag="bup")
                    ucol = ts * SEG - h * TSLAB
                    nc.tensor.matmul(
                        psums[ts][:],
                        bmat[k][:, nch * P:(nch + 1) * P],
                        u_t[k][:, ucol:ucol + SEG],
                        start=(k == 0), stop=(k == KH - 1),
                    )
            for ts in tsegs:
                if _is_fwd_chunk(nch):
                    nc.scalar.copy(v[nch][:, ts * SEG:(ts + 1) * SEG],
                                   psums[ts][:])
                else:
                    ss = NSEG - 1 - ts
                    dst = v[nch][:, ss * SEG:(ss + 1) * SEG]
                    nc.scalar.copy(dst[:, ::-1], psums[ts][:])

        for k in range(KH):
            nc.sync.dma_start(u_t[k][:], uT[k * P:(k + 1) * P, 0:TSLAB])
        for nch in g_half0:
            do_group(nch, 0)
        for k in range(KH):
            nc.sync.dma_start(u_t[k][:], uT[k * P:(k + 1) * P, TSLAB:L])
        for nch in g_half1:
            do_group(nch, 1)

        # ---- Phase B: twiddle-in on TSLAB slabs (in-place over v) ----
        # vr = c*br + s*bi ; vi = c*bi - s*br
        # mults on DVE; combines on GPSIMD for pairs {0,2}, DVE for {1,3}
        for tsl in range(NTSLAB):
            for c in C_ORDER:
                jre, jim = c, c + CCH
                sl = slice(tsl * TSLAB, (tsl + 1) * TSLAB)
                cs, sn = cosb[c][:, sl], sinb[c][:, sl]
                vre, vim = v[jre][:, sl], v[jim][:, sl]
                p1 = pscr.tile([P, TSLAB], F16, tag="p", name="p1")
                p2 = pscr.tile([P, TSLAB], F16, tag="p", name="p2")
                p3 = pscr.tile([P, TSLAB], F16, tag="p", name="p3")
                p4 = pscr.tile([P, TSLAB], F16, tag="p", name="p4")
                nc.vector.tensor_mul(p1[:], cs, vre)
                nc.vector.tensor_mul(p2[:], sn, vim)
                nc.vector.tensor_mul(p3[:], cs, vim)
                nc.vector.tensor_mul(p4[:], sn, vre)
                eng = nc.gpsimd if c in (0, 2) else nc.vector
                eng.tensor_add(vre, p1[:], p2[:])
                eng.tensor_sub(vim, p3[:], p4[:])

        # ---- Phases C/D per (scan-slab, complex chunk): scan + untwiddle ----
        # untwiddle overwrites the scanned slab, so the chain carry (last
        # column of w) is snapshotted into a tiny tile first
        carry_r = [None] * CCH
        carry_i = [None] * CCH
        for sb in range(NSLAB):
            for c in C_ORDER:
                jre, jim = c, c + CCH
                sl = slice(sb * SLAB, (sb + 1) * SLAB)
                cs, sn = cosb[c][:, sl], sinb[c][:, sl]
                vre, vim = v[jre][:, sl], v[jim][:, sl]

                # in-place scans over the v slab
                r_ap = rdec_t[:, c:c + 1].broadcast_to((P, SLAB))
                init_r = 0.0 if sb == 0 else carry_r[c][:, 0:1]
                init_i = 0.0 if sb == 0 else carry_i[c][:, 0:1]
                nc.vector.tensor_tensor_scan(vre, r_ap, vre, init_r,
                                             op0=MULT, op1=ADD)
                nc.vector.tensor_tensor_scan(vim, r_ap, vim, init_i,
                                             op0=MULT, op1=ADD)
                if sb < NSLAB - 1:
                    cr = crp.tile([P, 1], F16, tag="cr", name="crr")
                    ci = crp.tile([P, 1], F16, tag="cr", name="cri")
                    nc.vector.tensor_copy(cr[:], vre[:, SLAB - 1:SLAB])
                    nc.vector.tensor_copy(ci[:], vim[:, SLAB - 1:SLAB])
                    carry_r[c], carry_i[c] = cr, ci

                # untwiddle: xr = c*wr - s*wi ; xi = s*wr + c*wi
                q1 = qscr.tile([P, SLAB], F16, tag="q", name="q1")
                q2 = qscr.tile([P, SLAB], F16, tag="q", name="q2")
                q3 = qscr.tile([P, SLAB], F16, tag="q", name="q3")
                q4 = qscr.tile([P, SLAB], F16, tag="q", name="q4")
                nc.vector.tensor_mul(q1[:], cs, vre)
                nc.vector.tensor_mul(q2[:], sn, vim)
                nc.vector.tensor_mul(q3[:], sn, vre)
                nc.vector.tensor_mul(q4[:], cs, vim)
                if c < 2:
                    # forward: overwrite the spent v slab, t-order; GPSIMD
                    nc.gpsimd.tensor_sub(vre, q1[:], q2[:])
                    nc.gpsimd.tensor_add(vim, q3[:], q4[:])
                else:
                    # backward: reversed write into the spent cos/sin slab;
                    # t-slab (NSLAB-1-sb) content lands at table slab sb
                    nc.vector.tensor_sub(cs[:, ::-1], q1[:], q2[:])
                    nc.vector.tensor_add(sn[:, ::-1], q3[:], q4[:])

        # x source for the output matmul: real chunk k, time block i
        def x_src(k: int, i: int):
            if _is_fwd_chunk(k):
                return v[k][:, i * P:(i + 1) * P]
            j, o = divmod(i, SLAB // P)
            c = k % 4
            col = (NSLAB - 1 - j) * SLAB + o * P
            src = cosb[c] if k < 4 else sinb[c]
            return src[:, col:col + P]

        # ---- Phase E: y matmuls; t-slab readiness order [1,2,0,3] ----
        for sj in (1, 2, 0, 3):
            for o in range(SLAB // P):
                i = sj * (SLAB // P) + o
                py = yp.tile([P, H], F32, name="py", tag="yp")
                for k in range(NCH):
                    nc.tensor.matmul(
                        py[:], x_src(k, i), cmat[k][:],
                        start=(k == 0), stop=(k == NCH - 1),
                    )
                yt = ysb.tile([P, H], F32, tag="y", name="yt")
                nc.scalar.copy(yt[:], py[:])
                nc.sync.dma_start(y[i * P:(i + 1) * P, :], yt[:])

    nc.compile()
    return nc


def prepare_inputs(u, lengths, nu_log, theta_log, B, C):
    """Host-side prep: per-core in_maps. All heavy math in fp64 for accuracy."""
    u = np.asarray(u)
    lengths = np.asarray(lengths)
    nu = np.exp(np.asarray(nu_log, np.float64))
    theta = np.exp(np.asarray(theta_log, np.float64))
    r = np.exp(-nu)                                    # |lam|, (N,)

    j = np.arange(L, dtype=np.float64)
    ang = np.mod(theta[:, None] * j[None, :], 2 * np.pi)   # (N, L)
    cos_base = np.cos(ang).astype(np.float16)
    sin_base = np.sin(ang).astype(np.float16)

    Bcat = np.empty((H, 2 * N), np.float16)
    Bcat[:, :N] = np.asarray(B)[..., 0]
    Bcat[:, N:] = np.asarray(B)[..., 1]
    Ccat = np.empty((2 * N, H), np.float16)
    Ccat[:N] = np.asarray(C)[0]
    Ccat[N:] = -np.asarray(C)[1]
    rdec = r.reshape(CCH, P).T.astype(np.float32).copy()   # (128, 4)

    half = N // 2
    in_maps = []
    for b in range(BSZ):
        ln = int(lengths[b])
        ub = np.array(u[b], np.float32)
        if ln < L:
            ub[ln:, :] = 0.0
        uTh = np.ascontiguousarray(ub.T.astype(np.float16))
        cosb = cos_base.copy()
        sinb = sin_base.copy()
        if ln < L:
            cosb[:half, ln:] = 0
            sinb[:half, ln:] = 0
            cosb[half:, :L - ln] = 0
            sinb[half:, :L - ln] = 0
        in_maps.append({
            "uT": uTh, "cosT": cosb, "sinT": sinb,
            "rdec": rdec, "Bcat": Bcat, "Ccat": Ccat,
        })
    return in_maps


def kernel(u, lengths, nu_log, theta_log, B, C):
    global _CACHED
    from concourse.bass_utils import run_bass_kernel_spmd
    in_maps = prepare_inputs(u, lengths, nu_log, theta_log, B, C)
    if _CACHED is None:
        _CACHED = build_nc()
    res = run_bass_kernel_spmd(_CACHED, in_maps, list(range(BSZ)))
    y = np.stack([res.results[i]["y"] for i in range(BSZ)], axis=0)
    return y.astype(np.float32)


# revision 8
# speedup vs baseline: 1.1134x; 1.1134x over previous
"""Bidirectional complex-diagonal LRU (Linear Recurrent Unit) on 8 Trainium2 cores.

Math: lam = exp(-exp(nu_log) + i*exp(theta_log)) per channel n (N=512).
  Bu = einsum('blh,hn->bnl', u, B0 + iB1), masked to length.
  Forward scan over channels [0,256), backward (time-reversed) over [256,512).
  y = x.real @ C0 - x.imag @ C1, zeroed past each sequence length.

Device strategy (data-parallel, one batch per core):
  - Rotation trick: x_t = e^{i*th*t} * w_t turns the complex recurrence
    x_t = lam x_{t-1} + Bu_t into TWO real recurrences w_t = r w_{t-1} + v_t
    (r = |lam|), each a native DVE tensor_tensor_scan along the free dim.
  - Twiddle tables cos/sin(th*j) built on host in fp64 (exact phases), fp16 on
    device. Per-core masking (zero columns past the sequence length) is folded
    into the tables, so masking costs nothing on device.
  - Backward channels run on the reversed time axis; reversal happens inside
    the PSUM-evacuation copy (negative-stride AP) and the untwiddle writes.
  - All matmuls fp16 (full PE rate), accumulation in fp32 PSUM. Scans run
    in place over the v buffer; x overwrites v (fwd) / spent cos+sin table
    slices (bwd), so SBUF holds everything with no extra big buffers.
  - Elementwise combine ops are split DVE/GPSIMD to balance engine load.

Self-contained: hardcodes B=8, L=4096, H=N=512, 8 cores.
"""

import numpy as np
from contextlib import ExitStack

import concourse.bass as bass
import concourse.bacc as bacc
import concourse.mybir as mybir
import concourse.tile as tile

P = 128
L = 4096
H = 512
N = 512
BSZ = 8
SEG = 512                # Bu matmul / evac granularity (one PSUM bank)
NSEG = L // SEG          # 8
SLAB = 1024              # scan + untwiddle granularity
NSLAB = L // SLAB        # 4
TSLAB = 2048             # twiddle-in granularity
NTSLAB = L // TSLAB      # 2
KH = H // P              # 4 contraction chunks for Bu
NCH = 2 * N // P         # 8 real-channel chunks (re 0..3, im 4..7)
CCH = N // P             # 4 complex-channel chunks (0,1 fwd; 2,3 bwd)
NT = L // P              # 32 time blocks for the output matmul

F16 = mybir.dt.float16
F32 = mybir.dt.float32
MULT = mybir.AluOpType.mult
ADD = mybir.AluOpType.add

C_ORDER = [0, 2, 1, 3]

_CACHED = None


def _is_fwd_chunk(nch: int) -> bool:
    return (nch % 4) < 2


def build_nc():
    nc = bacc.Bacc("TRN2", target_bir_lowering=False, debug=False)
    uT = nc.declare_dram_parameter("uT", [H, L], F16, isOutput=False)
    cosT = nc.declare_dram_parameter("cosT", [N, L], F16, isOutput=False)
    sinT = nc.declare_dram_parameter("sinT", [N, L], F16, isOutput=False)
    rdec = nc.declare_dram_parameter("rdec", [P, CCH], F32, isOutput=False)
    Bcat = nc.declare_dram_parameter("Bcat", [H, 2 * N], F16, isOutput=False)
    Ccat = nc.declare_dram_parameter("Ccat", [2 * N, H], F16, isOutput=False)
    y = nc.declare_dram_parameter("y", [L, H], F32, isOutput=True)

    with tile.TileContext(nc) as tc, ExitStack() as ctx:
        const = ctx.enter_context(tc.tile_pool(name="const", bufs=1))
        big = ctx.enter_context(tc.tile_pool(name="big", bufs=1))
        upool = ctx.enter_context(tc.tile_pool(name="upool", bufs=1))
        pscr = ctx.enter_context(tc.tile_pool(name="pscr", bufs=6))
        qscr = ctx.enter_context(tc.tile_pool(name="qscr", bufs=8))
        ysb = ctx.enter_context(tc.tile_pool(name="ysb", bufs=3))
        crp = ctx.enter_context(tc.tile_pool(name="crp", bufs=16))
        bup = ctx.enter_context(tc.tile_pool(name="bup", bufs=6, space="PSUM"))
        yp = ctx.enter_context(tc.tile_pool(name="yp", bufs=2, space="PSUM"))

        # uT streamed in halves: cols [0:2048] then [2048:4096]
        u_t = [upool.tile([P, TSLAB], F16, tag=f"uT{k}", name=f"uT{k}")
               for k in range(KH)]
        cosb = [big.tile([P, L], F16, tag=f"cos{c}", name=f"cos{c}") for c in range(CCH)]
        sinb = [big.tile([P, L], F16, tag=f"sin{c}", name=f"sin{c}") for c in range(CCH)]
        v = [big.tile([P, L], F16, tag=f"v{j}", name=f"v{j}") for j in range(NCH)]
        bmat = [const.tile([P, 2 * N], F16, tag=f"B{k}", name=f"Bm{k}") for k in range(KH)]
        cmat = [const.tile([P, H], F16, tag=f"C{k}", name=f"Cm{k}") for k in range(NCH)]
        rdec_t = const.tile([P, CCH], F32, tag="rdec", name="rdec_t")

        # ---- constant DMAs ----
        nc.sync.dma_start(rdec_t[:], rdec[:])
        for k in range(KH):
            nc.sync.dma_start(bmat[k][:], Bcat[k * P:(k + 1) * P, :])
        for k in range(NCH):
            nc.sync.dma_start(cmat[k][:], Ccat[k * P:(k + 1) * P, :])
        for c in range(CCH):
            nc.sync.dma_start(cosb[c][:], cosT[c * P:(c + 1) * P, :])
            nc.sync.dma_start(sinb[c][:], sinT[c * P:(c + 1) * P, :])

        # ---- Phase A: Bu matmuls, evacuate into v slots (scan-time order) ----
        # uhalf 0 covers tsegs {0..3} (first-needed by fwd chunks), uhalf 1
        # covers tsegs {4..7} (first-needed by bwd chunks, reversed).  All
        # half-0 groups run first (uT tiles are reloaded for half 1); within
        # each half, chunk-pair priority interleave.
        g_half0 = [0, 4, 2, 6, 1, 5, 3, 7]
        g_half1 = [2, 6, 0, 4, 3, 7, 1, 5]

        def do_group(nch, h):
            tsegs = [0, 1, 2, 3] if h == 0 else [7, 6, 5, 4]
            for ts in tsegs:
                ps = bup.tile([P, SEG], F32, name=f"bups{ts}", tag="bup")
                ucol = ts * SEG - h * TSLAB
                for k in range(KH):
                    nc.tensor.matmul(
                        ps[:],
                        bmat[k][:, nch * P:(nch + 1) * P],
                        u_t[k][:, ucol:ucol + SEG],
                        start=(k == 0), stop=(k == KH - 1),
                    )
                if _is_fwd_chunk(nch):
                    nc.scalar.copy(v[nch][:, ts * SEG:(ts + 1) * SEG], ps[:])
                else:
                    ss = NSEG - 1 - ts
                    dst = v[nch][:, ss * SEG:(ss + 1) * SEG]
                    nc.scalar.copy(dst[:, ::-1], ps[:])

        for k in range(KH):
            nc.sync.dma_start(u_t[k][:], uT[k * P:(k + 1) * P, 0:TSLAB])
        for nch in g_half0:
            do_group(nch, 0)
        for k in range(KH):
            nc.sync.dma_start(u_t[k][:], uT[k * P:(k + 1) * P, TSLAB:L])
        for nch in g_half1:
            do_group(nch, 1)

        # ---- Phase B: twiddle-in on TSLAB slabs (in-place over v) ----
        # vr = c*br + s*bi ; vi = c*bi - s*br
        # mults on DVE; combines on GPSIMD for pairs {0,2}, DVE for {1,3}
        for tsl in range(NTSLAB):
            for c in C_ORDER:
                jre, jim = c, c + CCH
                sl = slice(tsl * TSLAB, (tsl + 1) * TSLAB)
                cs, sn = cosb[c][:, sl], sinb[c][:, sl]
                vre, vim = v[jre][:, sl], v[jim][:, sl]
                p1 = pscr.tile([P, TSLAB], F16, tag="p", name="p1")
                p2 = pscr.tile([P, TSLAB], F16, tag="p", name="p2")
                p3 = pscr.tile([P, TSLAB], F16, tag="p", name="p3")
                p4 = pscr.tile([P, TSLAB], F16, tag="p", name="p4")
                nc.vector.tensor_mul(p1[:], cs, vre)
                nc.vector.tensor_mul(p2[:], sn, vim)
                nc.vector.tensor_mul(p3[:], cs, vim)
                nc.vector.tensor_mul(p4[:], sn, vre)
                nc.vector.tensor_add(vre, p1[:], p2[:])
                nc.vector.tensor_sub(vim, p3[:], p4[:])

        # ---- Phases C/D per (scan-slab, complex chunk): scan + untwiddle ----
        # untwiddle overwrites the scanned slab, so the chain carry (last
        # column of w) is snapshotted into a tiny tile first
        carry_r = [None] * CCH
        carry_i = [None] * CCH
        for sb in range(NSLAB):
            for c in C_ORDER:
                jre, jim = c, c + CCH
                sl = slice(sb * SLAB, (sb + 1) * SLAB)
                cs, sn = cosb[c][:, sl], sinb[c][:, sl]
                vre, vim = v[jre][:, sl], v[jim][:, sl]

                # in-place scans over the v slab
                r_ap = rdec_t[:, c:c + 1].broadcast_to((P, SLAB))
                init_r = 0.0 if sb == 0 else carry_r[c][:, 0:1]
                init_i = 0.0 if sb == 0 else carry_i[c][:, 0:1]
                nc.vector.tensor_tensor_scan(vre, r_ap, vre, init_r,
                                             op0=MULT, op1=ADD)
                nc.vector.tensor_tensor_scan(vim, r_ap, vim, init_i,
                                             op0=MULT, op1=ADD)
                if sb < NSLAB - 1:
                    cr = crp.tile([P, 1], F16, tag="cr", name="crr")
                    ci = crp.tile([P, 1], F16, tag="cr", name="cri")
                    nc.vector.tensor_copy(cr[:], vre[:, SLAB - 1:SLAB])
                    nc.vector.tensor_copy(ci[:], vim[:, SLAB - 1:SLAB])
                    carry_r[c], carry_i[c] = cr, ci

                # untwiddle: xr = c*wr - s*wi ; xi = s*wr + c*wi
                q1 = qscr.tile([P, SLAB], F16, tag="q", name="q1")
                q2 = qscr.tile([P, SLAB], F16, tag="q", name="q2")
                q3 = qscr.tile([P, SLAB], F16, tag="q", name="q3")
                q4 = qscr.tile([P, SLAB], F16, tag="q", name="q4")
                nc.vector.tensor_mul(q1[:], cs, vre)
                nc.vector.tensor_mul(q2[:], sn, vim)
                nc.vector.tensor_mul(q3[:], sn, vre)
                nc.vector.tensor_mul(q4[:], cs, vim)
                if c < 2:
                    # forward: overwrite the spent v slab, t-order
                    nc.vector.tensor_sub(vre, q1[:], q2[:])
                    nc.vector.tensor_add(vim, q3[:], q4[:])
                else:
                    # backward: reversed write into the spent cos/sin slab;
                    # t-slab (NSLAB-1-sb) content lands at table slab sb
                    nc.vector.tensor_sub(cs[:, ::-1], q1[:], q2[:])
                    nc.vector.tensor_add(sn[:, ::-1], q3[:], q4[:])

        # x source for the output matmul: real chunk k, time block i
        def x_src(k: int, i: int):
            if _is_fwd_chunk(k):
                return v[k][:, i * P:(i + 1) * P]
            j, o = divmod(i, SLAB // P)
            c = k % 4
            col = (NSLAB - 1 - j) * SLAB + o * P
            src = cosb[c] if k < 4 else sinb[c]
            return src[:, col:col + P]

        # ---- Phase E: y matmuls; t-slab readiness order [1,2,0,3] ----
        for sj in (1, 2, 0, 3):
            for o in range(SLAB // P):
                i = sj * (SLAB // P) + o
                py = yp.tile([P, H], F32, name="py", tag="yp")
                for k in range(NCH):
                    nc.tensor.matmul(
                        py[:], x_src(k, i), cmat[k][:],
                        start=(k == 0), stop=(k == NCH - 1),
                    )
                yt = ysb.tile([P, H], F32, tag="y", name="yt")
                nc.scalar.copy(yt[:], py[:])
                nc.sync.dma_start(y[i * P:(i + 1) * P, :], yt[:])

    nc.compile()
    return nc


def prepare_inputs(u, lengths, nu_log, theta_log, B, C):
    """Host-side prep: per-core in_maps. All heavy math in fp64 for accuracy."""
    u = np.asarray(u)
    lengths = np.asarray(lengths)
    nu = np.exp(np.asarray(nu_log, np.float64))
    theta = np.exp(np.asarray(theta_log, np.float64))
    r = np.exp(-nu)                                    # |lam|, (N,)

    j = np.arange(L, dtype=np.float64)
    ang = np.mod(theta[:, None] * j[None, :], 2 * np.pi)   # (N, L)
    cos_base = np.cos(ang).astype(np.float16)
    sin_base = np.sin(ang).astype(np.float16)

    Bcat = np.empty((H, 2 * N), np.float16)
    Bcat[:, :N] = np.asarray(B)[..., 0]
    Bcat[:, N:] = np.asarray(B)[..., 1]
    Ccat = np.empty((2 * N, H), np.float16)
    Ccat[:N] = np.asarray(C)[0]
    Ccat[N:] = -np.asarray(C)[1]
    rdec = r.reshape(CCH, P).T.astype(np.float32).copy()   # (128, 4)

    half = N // 2
    in_maps = []
    for b in range(BSZ):
        ln = int(lengths[b])
        ub = np.array(u[b], np.float32)
        if ln < L:
            ub[ln:, :] = 0.0
        uTh = np.ascontiguousarray(ub.T.astype(np.float16))
        cosb = cos_base.copy()
        sinb = sin_base.copy()
        if ln < L:
            cosb[:half, ln:] = 0
            sinb[:half, ln:] = 0
            cosb[half:, :L - ln] = 0
            sinb[half:, :L - ln] = 0
        in_maps.append({
            "uT": uTh, "cosT": cosb, "sinT": sinb,
            "rdec": rdec, "Bcat": Bcat, "Ccat": Ccat,
        })
    return in_maps


def kernel(u, lengths, nu_log, theta_log, B, C):
    global _CACHED
    from concourse.bass_utils import run_bass_kernel_spmd
    in_maps = prepare_inputs(u, lengths, nu_log, theta_log, B, C)
    if _CACHED is None:
        _CACHED = build_nc()
    res = run_bass_kernel_spmd(_CACHED, in_maps, list(range(BSZ)))
    y = np.stack([res.results[i]["y"] for i in range(BSZ)], axis=0)
    return y.astype(np.float32)


# revision 11
# speedup vs baseline: 1.2024x; 1.0800x over previous
"""Bidirectional complex-diagonal LRU (Linear Recurrent Unit) on 8 Trainium2 cores.

Math: lam = exp(-exp(nu_log) + i*exp(theta_log)) per channel n (N=512).
  Bu = einsum('blh,hn->bnl', u, B0 + iB1), masked to length.
  Forward scan over channels [0,256), backward (time-reversed) over [256,512).
  y = x.real @ C0 - x.imag @ C1, zeroed past each sequence length.

Device strategy (data-parallel, one batch per core):
  - Rotation trick: x_t = e^{i*th*t} * w_t turns the complex recurrence
    x_t = lam x_{t-1} + Bu_t into TWO real recurrences w_t = r w_{t-1} + v_t
    (r = |lam|), each a native DVE tensor_tensor_scan along the free dim.
  - Twiddle tables cos/sin(th*j) built on host in fp64 (exact phases), fp16 on
    device. Per-core masking (zero columns past the sequence length) is folded
    into the tables, so masking costs nothing on device.
  - Backward channels run on the reversed time axis; reversal happens inside
    the PSUM-evacuation copy (negative-stride AP) and the untwiddle writes.
  - All matmuls fp16 (full PE rate), accumulation in fp32 PSUM. Scans run
    in place over the v buffer; x overwrites v (fwd) / spent cos+sin table
    slices (bwd), so SBUF holds everything with no extra big buffers.
  - Elementwise combine ops are split DVE/GPSIMD to balance engine load.

Self-contained: hardcodes B=8, L=4096, H=N=512, 8 cores.
"""

import numpy as np
from contextlib import ExitStack

import concourse.bass as bass
import concourse.bacc as bacc
import concourse.mybir as mybir
import concourse.tile as tile

P = 128
L = 4096
H = 512
N = 512
BSZ = 8
SEG = 512                # Bu matmul / evac granularity (one PSUM bank)
NSEG = L // SEG          # 8
SLAB = 1024              # scan + untwiddle granularity
NSLAB = L // SLAB        # 4
TSLAB = 2048             # twiddle-in granularity
NTSLAB = L // TSLAB      # 2
KH = H // P              # 4 contraction chunks for Bu
NCH = 2 * N // P         # 8 real-channel chunks (re 0..3, im 4..7)
CCH = N // P             # 4 complex-channel chunks (0,1 fwd; 2,3 bwd)
NT = L // P              # 32 time blocks for the output matmul

F16 = mybir.dt.float16
F32 = mybir.dt.float32
MULT = mybir.AluOpType.mult
ADD = mybir.AluOpType.add

C_ORDER = [0, 2, 1, 3]

_CACHED = None


def _is_fwd_chunk(nch: int) -> bool:
    return (nch % 4) < 2


def build_nc():
    nc = bacc.Bacc("TRN2", target_bir_lowering=False, debug=False)
    uT = nc.declare_dram_parameter("uT", [H, L], F16, isOutput=False)
    cosT = nc.declare_dram_parameter("cosT", [N, L], F16, isOutput=False)
    sinT = nc.declare_dram_parameter("sinT", [N, L], F16, isOutput=False)
    rdec = nc.declare_dram_parameter("rdec", [P, CCH], F32, isOutput=False)
    Bcat = nc.declare_dram_parameter("Bcat", [H, 2 * N], F16, isOutput=False)
    Ccat = nc.declare_dram_parameter("Ccat", [2 * N, H], F16, isOutput=False)
    y = nc.declare_dram_parameter("y", [L, H], F32, isOutput=True)

    with tile.TileContext(nc) as tc, ExitStack() as ctx:
        const = ctx.enter_context(tc.tile_pool(name="const", bufs=1))
        big = ctx.enter_context(tc.tile_pool(name="big", bufs=1))
        upool = ctx.enter_context(tc.tile_pool(name="upool", bufs=1))
        pscr = ctx.enter_context(tc.tile_pool(name="pscr", bufs=5))
        qscr = ctx.enter_context(tc.tile_pool(name="qscr", bufs=8))
        wpool = ctx.enter_context(tc.tile_pool(name="wpool", bufs=10))
        ysb = ctx.enter_context(tc.tile_pool(name="ysb", bufs=3))
        bup = ctx.enter_context(tc.tile_pool(name="bup", bufs=6, space="PSUM"))
        yp = ctx.enter_context(tc.tile_pool(name="yp", bufs=2, space="PSUM"))

        # uT streamed in halves: cols [0:2048] then [2048:4096]
        u_t = [upool.tile([P, TSLAB], F16, tag=f"uT{k}", name=f"uT{k}")
               for k in range(KH)]
        cosb = [big.tile([P, L], F16, tag=f"cos{c}", name=f"cos{c}") for c in range(CCH)]
        sinb = [big.tile([P, L], F16, tag=f"sin{c}", name=f"sin{c}") for c in range(CCH)]
        v = [big.tile([P, L], F16, tag=f"v{j}", name=f"v{j}") for j in range(NCH)]
        bmat = [const.tile([P, 2 * N], F16, tag=f"B{k}", name=f"Bm{k}") for k in range(KH)]
        cmat = [const.tile([P, H], F16, tag=f"C{k}", name=f"Cm{k}") for k in range(NCH)]
        rdec_t = const.tile([P, CCH], F32, tag="rdec", name="rdec_t")

        # ---- constant DMAs (uT half 0 + Bcat first so Bu starts ASAP;
        #      tables next for twiddle-in; Ccat last, needed only by phase E)
        nc.sync.dma_start(rdec_t[:], rdec[:])
        for k in range(KH):
            nc.sync.dma_start(bmat[k][:], Bcat[k * P:(k + 1) * P, :])
        for k in range(KH):
            nc.sync.dma_start(u_t[k][:], uT[k * P:(k + 1) * P, 0:TSLAB])
        for c in range(CCH):
            nc.sync.dma_start(cosb[c][:], cosT[c * P:(c + 1) * P, :])
            nc.sync.dma_start(sinb[c][:], sinT[c * P:(c + 1) * P, :])
        for k in range(NCH):
            nc.sync.dma_start(cmat[k][:], Ccat[k * P:(k + 1) * P, :])

        # ---- Phase A: Bu matmuls, evacuate into v slots (scan-time order) ----
        # uhalf 0 covers tsegs {0..3} (first-needed by fwd chunks), uhalf 1
        # covers tsegs {4..7} (first-needed by bwd chunks, reversed).  All
        # half-0 groups run first (uT tiles are reloaded for half 1); within
        # each half, chunk-pair priority interleave.
        g_half0 = [0, 4, 2, 6, 1, 5, 3, 7]
        g_half1 = [2, 6, 0, 4, 3, 7, 1, 5]

        def do_group(nch, h):
            tsegs = [0, 1, 2, 3] if h == 0 else [7, 6, 5, 4]
            for ts in tsegs:
                ps = bup.tile([P, SEG], F32, name=f"bups{ts}", tag="bup")
                ucol = ts * SEG - h * TSLAB
                for k in range(KH):
                    nc.tensor.matmul(
                        ps[:],
                        bmat[k][:, nch * P:(nch + 1) * P],
                        u_t[k][:, ucol:ucol + SEG],
                        start=(k == 0), stop=(k == KH - 1),
                    )
                if _is_fwd_chunk(nch):
                    nc.scalar.copy(v[nch][:, ts * SEG:(ts + 1) * SEG], ps[:])
                else:
                    ss = NSEG - 1 - ts
                    dst = v[nch][:, ss * SEG:(ss + 1) * SEG]
                    nc.scalar.copy(dst[:, ::-1], ps[:])

        for nch in g_half0:
            do_group(nch, 0)
        for k in range(KH):
            nc.sync.dma_start(u_t[k][:], uT[k * P:(k + 1) * P, TSLAB:L])
        for nch in g_half1:
            do_group(nch, 1)

        # ---- Phase B: twiddle-in on TSLAB slabs (in-place over v) ----
        # vr = c*br + s*bi ; vi = c*bi - s*br
        # mults on DVE; combines on GPSIMD for pairs {0,2}, DVE for {1,3}
        for tsl in range(NTSLAB):
            for c in C_ORDER:
                jre, jim = c, c + CCH
                sl = slice(tsl * TSLAB, (tsl + 1) * TSLAB)
                cs, sn = cosb[c][:, sl], sinb[c][:, sl]
                vre, vim = v[jre][:, sl], v[jim][:, sl]
                p1 = pscr.tile([P, TSLAB], F16, tag="p", name="p1")
                p2 = pscr.tile([P, TSLAB], F16, tag="p", name="p2")
                p3 = pscr.tile([P, TSLAB], F16, tag="p", name="p3")
                p4 = pscr.tile([P, TSLAB], F16, tag="p", name="p4")
                nc.vector.tensor_mul(p1[:], cs, vre)
                nc.vector.tensor_mul(p2[:], sn, vim)
                nc.vector.tensor_mul(p3[:], cs, vim)
                nc.vector.tensor_mul(p4[:], sn, vre)
                nc.vector.tensor_add(vre, p1[:], p2[:])
                nc.vector.tensor_sub(vim, p3[:], p4[:])

        # ---- Phases C/D per (scan-seg 512, complex chunk): scan + untwiddle ----
        # scans write to a small w ring (out of place); untwiddle consumes w
        # and writes x over the spent v seg (fwd) / cos+sin seg (bwd, scan
        # order -- the y matmul reads those with reversed lhsT columns)
        prev_w = [None] * (2 * CCH)
        for ss in range(NSEG):
            for c in C_ORDER:
                jre, jim = c, c + CCH
                sl = slice(ss * SEG, (ss + 1) * SEG)
                cs, sn = cosb[c][:, sl], sinb[c][:, sl]
                vre, vim = v[jre][:, sl], v[jim][:, sl]

                r_ap = rdec_t[:, c:c + 1].broadcast_to((P, SEG))
                init_r = 0.0 if ss == 0 else prev_w[jre][:, SEG - 1:SEG]
                init_i = 0.0 if ss == 0 else prev_w[jim][:, SEG - 1:SEG]
                wr = wpool.tile([P, SEG], F16, tag="w", name="wr")
                wi = wpool.tile([P, SEG], F16, tag="w", name="wi")
                nc.vector.tensor_tensor_scan(wr[:], r_ap, vre, init_r,
                                             op0=MULT, op1=ADD)
                nc.vector.tensor_tensor_scan(wi[:], r_ap, vim, init_i,
                                             op0=MULT, op1=ADD)
                prev_w[jre], prev_w[jim] = wr, wi

                # untwiddle: xr = c*wr - s*wi ; xi = s*wr + c*wi
                q1 = qscr.tile([P, SEG], F16, tag="q", name="q1")
                q2 = qscr.tile([P, SEG], F16, tag="q", name="q2")
                q3 = qscr.tile([P, SEG], F16, tag="q", name="q3")
                q4 = qscr.tile([P, SEG], F16, tag="q", name="q4")
                nc.vector.tensor_mul(q1[:], cs, wr[:])
                nc.vector.tensor_mul(q2[:], sn, wi[:])
                nc.vector.tensor_mul(q3[:], sn, wr[:])
                nc.vector.tensor_mul(q4[:], cs, wi[:])
                if c < 2:
                    nc.vector.tensor_sub(vre, q1[:], q2[:])
                    nc.vector.tensor_add(vim, q3[:], q4[:])
                else:
                    # reversed write: t-seg (NSEG-1-ss) lands at table seg ss
                    nc.vector.tensor_sub(cs[:, ::-1], q1[:], q2[:])
                    nc.vector.tensor_add(sn[:, ::-1], q3[:], q4[:])

        # x source for the output matmul: real chunk k, time block i.
        # bwd x: t-seg (i//4) lives at table seg (NSEG-1-i//4), t-ordered.
        def x_src(k: int, i: int):
            if _is_fwd_chunk(k):
                return v[k][:, i * P:(i + 1) * P]
            c = k % 4
            col = (NSEG - 1 - i // 4) * SEG + (i % 4) * P
            src = cosb[c] if k < 4 else sinb[c]
            return src[:, col:col + P]

        # ---- Phase E: y matmuls; t-seg readiness order ----
        for sj in (3, 4, 2, 5, 1, 6, 0, 7):
            for o in range(SEG // P):
                i = sj * (SEG // P) + o
                py = yp.tile([P, H], F32, name="py", tag="yp")
                for k in range(NCH):
                    nc.tensor.matmul(
                        py[:], x_src(k, i), cmat[k][:],
                        start=(k == 0), stop=(k == NCH - 1),
                    )
                yt = ysb.tile([P, H], F32, tag="y", name="yt")
                nc.scalar.copy(yt[:], py[:])
                nc.sync.dma_start(y[i * P:(i + 1) * P, :], yt[:])

    nc.compile()
    return nc


def prepare_inputs(u, lengths, nu_log, theta_log, B, C):
    """Host-side prep: per-core in_maps. All heavy math in fp64 for accuracy."""
    u = np.asarray(u)
    lengths = np.asarray(lengths)
    nu = np.exp(np.asarray(nu_log, np.float64))
    theta = np.exp(np.asarray(theta_log, np.float64))
    r = np.exp(-nu)                                    # |lam|, (N,)

    j = np.arange(L, dtype=np.float64)
    ang = np.mod(theta[:, None] * j[None, :], 2 * np.pi)   # (N, L)
    cos_base = np.cos(ang).astype(np.float16)
    sin_base = np.sin(ang).astype(np.float16)

    Bcat = np.empty((H, 2 * N), np.float16)
    Bcat[:, :N] = np.asarray(B)[..., 0]
    Bcat[:, N:] = np.asarray(B)[..., 1]
    Ccat = np.empty((2 * N, H), np.float16)
    Ccat[:N] = np.asarray(C)[0]
    Ccat[N:] = -np.asarray(C)[1]
    rdec = r.reshape(CCH, P).T.astype(np.float32).copy()   # (128, 4)

    half = N // 2
    in_maps = []
    for b in range(BSZ):
        ln = int(lengths[b])
        ub = np.array(u[b], np.float32)
        if ln < L:
            ub[ln:, :] = 0.0
        uTh = np.ascontiguousarray(ub.T.astype(np.float16))
        cosb = cos_base.copy()
        sinb = sin_base.copy()
        if ln < L:
            cosb[:half, ln:] = 0
            sinb[:half, ln:] = 0
            cosb[half:, :L - ln] = 0
            sinb[half:, :L - ln] = 0
        in_maps.append({
            "uT": uTh, "cosT": cosb, "sinT": sinb,
            "rdec": rdec, "Bcat": Bcat, "Ccat": Ccat,
        })
    return in_maps


def kernel(u, lengths, nu_log, theta_log, B, C):
    global _CACHED
    from concourse.bass_utils import run_bass_kernel_spmd
    in_maps = prepare_inputs(u, lengths, nu_log, theta_log, B, C)
    if _CACHED is None:
        _CACHED = build_nc()
    res = run_bass_kernel_spmd(_CACHED, in_maps, list(range(BSZ)))
    y = np.stack([res.results[i]["y"] for i in range(BSZ)], axis=0)
    return y.astype(np.float32)


# revision 15
# speedup vs baseline: 1.3154x; 1.0940x over previous
"""Bidirectional complex-diagonal LRU (Linear Recurrent Unit) on 8 Trainium2 cores.

Math: lam = exp(-exp(nu_log) + i*exp(theta_log)) per channel n (N=512).
  Bu = einsum('blh,hn->bnl', u, B0 + iB1), masked to length.
  Forward scan over channels [0,256), backward (time-reversed) over [256,512).
  y = x.real @ C0 - x.imag @ C1, zeroed past each sequence length.

Device strategy (data-parallel, one batch per core):
  - Rotation trick: x_t = e^{i*th*t} * w_t turns the complex recurrence
    x_t = lam x_{t-1} + Bu_t into TWO real recurrences w_t = r w_{t-1} + v_t
    (r = |lam|), each a native DVE tensor_tensor_scan along the free dim.
  - Twiddle tables cos/sin(th*j) built on host in fp64 (exact phases), fp16 on
    device. Per-core masking (zero columns past the sequence length) is folded
    into the tables, so masking costs nothing on device.
  - Backward channels run on the reversed time axis; reversal happens inside
    the PSUM-evacuation copy (negative-stride AP) and the untwiddle writes.
  - All matmuls fp16 (full PE rate), accumulation in fp32 PSUM. Scans run
    in place over the v buffer; x overwrites v (fwd) / spent cos+sin table
    slices (bwd), so SBUF holds everything with no extra big buffers.
  - Elementwise combine ops are split DVE/GPSIMD to balance engine load.

Self-contained: hardcodes B=8, L=4096, H=N=512, 8 cores.
"""

import numpy as np
from contextlib import ExitStack

import concourse.bass as bass
import concourse.bacc as bacc
import concourse.mybir as mybir
import concourse.tile as tile

P = 128
L = 4096
H = 512
N = 512
BSZ = 8
SEG = 512                # Bu matmul / evac granularity (one PSUM bank)
NSEG = L // SEG          # 8
SLAB = 1024              # scan + untwiddle granularity
NSLAB = L // SLAB        # 4
TSLAB = 2048             # twiddle-in granularity
NTSLAB = L // TSLAB      # 2
KH = H // P              # 4 contraction chunks for Bu
NCH = 2 * N // P         # 8 real-channel chunks (re 0..3, im 4..7)
CCH = N // P             # 4 complex-channel chunks (0,1 fwd; 2,3 bwd)
NT = L // P              # 32 time blocks for the output matmul

F16 = mybir.dt.float16
F32 = mybir.dt.float32
MULT = mybir.AluOpType.mult
ADD = mybir.AluOpType.add

C_ORDER = [0, 2, 1, 3]

_CACHED = None


def _is_fwd_chunk(nch: int) -> bool:
    return (nch % 4) < 2


def build_nc():
    nc = bacc.Bacc("TRN2", target_bir_lowering=False, debug=False)
    uT = nc.declare_dram_parameter("uT", [H, L], F16, isOutput=False)
    cosT = nc.declare_dram_parameter("cosT", [N, L], F16, isOutput=False)
    sinT = nc.declare_dram_parameter("sinT", [N, L], F16, isOutput=False)
    rdec = nc.declare_dram_parameter("rdec", [P, CCH], F32, isOutput=False)
    Bcat = nc.declare_dram_parameter("Bcat", [H, 2 * N], F16, isOutput=False)
    Ccat = nc.declare_dram_parameter("Ccat", [2 * N, H], F16, isOutput=False)
    y = nc.declare_dram_parameter("y", [L, H], F32, isOutput=True)

    with tile.TileContext(nc) as tc, ExitStack() as ctx:
        const = ctx.enter_context(tc.tile_pool(name="const", bufs=1))
        big = ctx.enter_context(tc.tile_pool(name="big", bufs=1))
        upool = ctx.enter_context(tc.tile_pool(name="upool", bufs=1))
        pscr = ctx.enter_context(tc.tile_pool(name="pscr", bufs=4))
        qscr = ctx.enter_context(tc.tile_pool(name="qscr", bufs=6))
        wpool = ctx.enter_context(tc.tile_pool(name="wpool", bufs=7))
        ysb = ctx.enter_context(tc.tile_pool(name="ysb", bufs=2))
        bup = ctx.enter_context(tc.tile_pool(name="bup", bufs=6, space="PSUM"))
        yp = ctx.enter_context(tc.tile_pool(name="yp", bufs=2, space="PSUM"))

        # uT streamed in halves: cols [0:2048] then [2048:4096]
        u_t = [upool.tile([P, TSLAB], F16, tag=f"uT{k}", name=f"uT{k}")
               for k in range(KH)]
        cosb = [big.tile([P, L], F16, tag=f"cos{c}", name=f"cos{c}") for c in range(CCH)]
        sinb = [big.tile([P, L], F16, tag=f"sin{c}", name=f"sin{c}") for c in range(CCH)]
        v = [big.tile([P, L], F16, tag=f"v{j}", name=f"v{j}") for j in range(NCH)]
        bmat = [const.tile([P, 2 * N], F16, tag=f"B{k}", name=f"Bm{k}") for k in range(KH)]
        cmat = [const.tile([P, H], F16, tag=f"C{k}", name=f"Cm{k}") for k in range(NCH)]
        rdec_t = const.tile([P, CCH], F32, tag="rdec", name="rdec_t")

        # ---- constant DMAs (uT half 0 + Bcat first so Bu starts ASAP;
        #      tables next for twiddle-in; Ccat last, needed only by phase E)
        nc.sync.dma_start(rdec_t[:], rdec[:])
        for k in range(KH):
            nc.sync.dma_start(bmat[k][:], Bcat[k * P:(k + 1) * P, :])
        for k in range(KH):
            nc.sync.dma_start(u_t[k][:], uT[k * P:(k + 1) * P, 0:TSLAB])
        for c in range(CCH):
            nc.sync.dma_start(cosb[c][:], cosT[c * P:(c + 1) * P, :])
            nc.sync.dma_start(sinb[c][:], sinT[c * P:(c + 1) * P, :])
        for k in range(NCH):
            nc.sync.dma_start(cmat[k][:], Ccat[k * P:(k + 1) * P, :])

        # ---- Phase A: Bu matmuls, evacuate into v slots (scan-time order) ----
        # uhalf 0 covers tsegs {0..3} (first-needed by fwd chunks), uhalf 1
        # covers tsegs {4..7} (first-needed by bwd chunks, reversed).  All
        # half-0 groups run first (uT tiles are reloaded for half 1); within
        # each half, chunk-pair priority interleave.
        g_half0 = [0, 4, 2, 6, 1, 5, 3, 7]
        g_half1 = [2, 6, 0, 4, 3, 7, 1, 5]

        def do_group(nch, h):
            tsegs = [0, 1, 2, 3] if h == 0 else [7, 6, 5, 4]
            for ts in tsegs:
                ps = bup.tile([P, SEG], F32, name=f"bups{ts}", tag="bup")
                ucol = ts * SEG - h * TSLAB
                for k in range(KH):
                    nc.tensor.matmul(
                        ps[:],
                        bmat[k][:, nch * P:(nch + 1) * P],
                        u_t[k][:, ucol:ucol + SEG],
                        start=(k == 0), stop=(k == KH - 1),
                    )
                if _is_fwd_chunk(nch):
                    nc.scalar.copy(v[nch][:, ts * SEG:(ts + 1) * SEG], ps[:])
                else:
                    ss = NSEG - 1 - ts
                    dst = v[nch][:, ss * SEG:(ss + 1) * SEG]
                    nc.scalar.copy(dst[:, ::-1], ps[:])

        for nch in g_half0:
            do_group(nch, 0)
        for k in range(KH):
            nc.sync.dma_start(u_t[k][:], uT[k * P:(k + 1) * P, TSLAB:L])
        for nch in g_half1:
            do_group(nch, 1)

        # ---- Phases B/C/D: twiddle-in (2048), scan+untwiddle (1024) ----
        # Emitted in dependency-feasible stream order (DVE executes in order):
        #  S1: needs only uT half 0  -> fwd chunks, scan-slabs 0..1
        #  S2: needs half 1          -> bwd slabs 0..1 + fwd slabs 2..3
        #  S3: bwd slabs 2..3
        prev_w = [None] * (2 * CCH)

        def twiddle_in(c, tsl):
            jre, jim = c, c + CCH
            sl = slice(tsl * TSLAB, (tsl + 1) * TSLAB)
            cs, sn = cosb[c][:, sl], sinb[c][:, sl]
            vre, vim = v[jre][:, sl], v[jim][:, sl]
            p1 = pscr.tile([P, TSLAB], F16, tag="p", name="p1")
            p2 = pscr.tile([P, TSLAB], F16, tag="p", name="p2")
            p3 = pscr.tile([P, TSLAB], F16, tag="p", name="p3")
            p4 = pscr.tile([P, TSLAB], F16, tag="p", name="p4")
            nc.vector.tensor_mul(p1[:], cs, vre)
            nc.vector.tensor_mul(p2[:], sn, vim)
            nc.vector.tensor_mul(p3[:], cs, vim)
            nc.vector.tensor_mul(p4[:], sn, vre)
            nc.vector.tensor_add(vre, p1[:], p2[:])
            nc.vector.tensor_sub(vim, p3[:], p4[:])

        def scan_untw(c, sb):
            jre, jim = c, c + CCH
            sl = slice(sb * SLAB, (sb + 1) * SLAB)
            cs, sn = cosb[c][:, sl], sinb[c][:, sl]
            vre, vim = v[jre][:, sl], v[jim][:, sl]
            r_ap = rdec_t[:, c:c + 1].broadcast_to((P, SLAB))
            init_r = 0.0 if sb == 0 else prev_w[jre][:, SLAB - 1:SLAB]
            init_i = 0.0 if sb == 0 else prev_w[jim][:, SLAB - 1:SLAB]
            wr = wpool.tile([P, SLAB], F16, tag="w", name="wr")
            wi = wpool.tile([P, SLAB], F16, tag="w", name="wi")
            nc.vector.tensor_tensor_scan(wr[:], r_ap, vre, init_r,
                                         op0=MULT, op1=ADD)
            nc.vector.tensor_tensor_scan(wi[:], r_ap, vim, init_i,
                                         op0=MULT, op1=ADD)
            prev_w[jre], prev_w[jim] = wr, wi

            q1 = qscr.tile([P, SLAB], F16, tag="q", name="q1")
            q2 = qscr.tile([P, SLAB], F16, tag="q", name="q2")
            q3 = qscr.tile([P, SLAB], F16, tag="q", name="q3")
            q4 = qscr.tile([P, SLAB], F16, tag="q", name="q4")
            nc.vector.tensor_mul(q1[:], cs, wr[:])
            nc.vector.tensor_mul(q2[:], sn, wi[:])
            nc.vector.tensor_mul(q3[:], sn, wr[:])
            nc.vector.tensor_mul(q4[:], cs, wi[:])
            if c < 2:
                nc.vector.tensor_sub(vre, q1[:], q2[:])
                nc.vector.tensor_add(vim, q3[:], q4[:])
            else:
                # reversed write: t-slab (NSLAB-1-sb) lands at table slab sb
                nc.vector.tensor_sub(cs[:, ::-1], q1[:], q2[:])
                nc.vector.tensor_add(sn[:, ::-1], q3[:], q4[:])

        # S1
        twiddle_in(0, 0)
        twiddle_in(1, 0)
        for sb in (0, 1):
            for c in (0, 1):
                scan_untw(c, sb)
        # S2: fwd slabs 2..3 first (completes y slab 3), then bwd slabs 0..1
        twiddle_in(0, 1)
        twiddle_in(1, 1)
        twiddle_in(2, 0)
        twiddle_in(3, 0)
        scan_untw(0, 2)
        scan_untw(0, 3)
        scan_untw(1, 2)
        scan_untw(1, 3)
        scan_untw(2, 0)
        scan_untw(2, 1)
        scan_untw(3, 0)
        scan_untw(3, 1)
        # S3: bwd slabs 2..3 (completes y slabs 1 then 0)
        twiddle_in(2, 1)
        twiddle_in(3, 1)
        scan_untw(2, 2)
        scan_untw(2, 3)
        scan_untw(3, 2)
        scan_untw(3, 3)

        # x source for the output matmul: real chunk k, time block i.
        # bwd x: t-slab (i//8) lives at table slab (NSLAB-1-i//8), t-ordered.
        def x_src(k: int, i: int):
            if _is_fwd_chunk(k):
                return v[k][:, i * P:(i + 1) * P]
            c = k % 4
            col = (NSLAB - 1 - i // 8) * SLAB + (i % 8) * P
            src = cosb[c] if k < 4 else sinb[c]
            return src[:, col:col + P]

        # ---- Phase E: y matmuls; t-slab readiness order [3,2,1,0] ----
        for sj in (3, 2, 1, 0):
            for o in range(SLAB // P):
                i = sj * (SLAB // P) + o
                py = yp.tile([P, H], F32, name="py", tag="yp")
                for k in range(NCH):
                    nc.tensor.matmul(
                        py[:], x_src(k, i), cmat[k][:],
                        start=(k == 0), stop=(k == NCH - 1),
                    )
                yt = ysb.tile([P, H], F32, tag="y", name="yt")
                nc.scalar.copy(yt[:], py[:])
                nc.sync.dma_start(y[i * P:(i + 1) * P, :], yt[:])

    nc.compile()
    return nc


def prepare_inputs(u, lengths, nu_log, theta_log, B, C):
    """Host-side prep: per-core in_maps. All heavy math in fp64 for accuracy."""
    u = np.asarray(u)
    lengths = np.asarray(lengths)
    nu = np.exp(np.asarray(nu_log, np.float64))
    theta = np.exp(np.asarray(theta_log, np.float64))
    r = np.exp(-nu)                                    # |lam|, (N,)

    j = np.arange(L, dtype=np.float64)
    ang = np.mod(theta[:, None] * j[None, :], 2 * np.pi)   # (N, L)
    cos_base = np.cos(ang).astype(np.float16)
    sin_base = np.sin(ang).astype(np.float16)

    Bcat = np.empty((H, 2 * N), np.float16)
    Bcat[:, :N] = np.asarray(B)[..., 0]
    Bcat[:, N:] = np.asarray(B)[..., 1]
    Ccat = np.empty((2 * N, H), np.float16)
    Ccat[:N] = np.asarray(C)[0]
    Ccat[N:] = -np.asarray(C)[1]
    rdec = r.reshape(CCH, P).T.astype(np.float32).copy()   # (128, 4)

    half = N // 2
    in_maps = []
    for b in range(BSZ):
        ln = int(lengths[b])
        ub = np.array(u[b], np.float32)
        if ln < L:
            ub[ln:, :] = 0.0
        uTh = np.ascontiguousarray(ub.T.astype(np.float16))
        cosb = cos_base.copy()
        sinb = sin_base.copy()
        if ln < L:
            cosb[:half, ln:] = 0
            sinb[:half, ln:] = 0
            cosb[half:, :L - ln] = 0
            sinb[half:, :L - ln] = 0
        in_maps.append({
            "uT": uTh, "cosT": cosb, "sinT": sinb,
            "rdec": rdec, "Bcat": Bcat, "Ccat": Ccat,
        })
    return in_maps


def kernel(u, lengths, nu_log, theta_log, B, C):
    global _CACHED
    from concourse.bass_utils import run_bass_kernel_spmd
    in_maps = prepare_inputs(u, lengths, nu_log, theta_log, B, C)
    if _CACHED is None:
        _CACHED = build_nc()
    res = run_bass_kernel_spmd(_CACHED, in_maps, list(range(BSZ)))
    y = np.stack([res.results[i]["y"] for i in range(BSZ)], axis=0)
    return y.astype(np.float32)


# revision 16
# speedup vs baseline: 1.3341x; 1.0142x over previous
"""Bidirectional complex-diagonal LRU (Linear Recurrent Unit) on 8 Trainium2 cores.

Math: lam = exp(-exp(nu_log) + i*exp(theta_log)) per channel n (N=512).
  Bu = einsum('blh,hn->bnl', u, B0 + iB1), masked to length.
  Forward scan over channels [0,256), backward (time-reversed) over [256,512).
  y = x.real @ C0 - x.imag @ C1, zeroed past each sequence length.

Device strategy (data-parallel, one batch per core):
  - Rotation trick: x_t = e^{i*th*t} * w_t turns the complex recurrence
    x_t = lam x_{t-1} + Bu_t into TWO real recurrences w_t = r w_{t-1} + v_t
    (r = |lam|), each a native DVE tensor_tensor_scan along the free dim.
  - Twiddle tables cos/sin(th*j) built on host in fp64 (exact phases), fp16 on
    device. Per-core masking (zero columns past the sequence length) is folded
    into the tables, so masking costs nothing on device.
  - Backward channels run on the reversed time axis; reversal happens inside
    the PSUM-evacuation copy (negative-stride AP) and the untwiddle writes.
  - All matmuls fp16 (full PE rate), accumulation in fp32 PSUM. Scans run
    in place over the v buffer; x overwrites v (fwd) / spent cos+sin table
    slices (bwd), so SBUF holds everything with no extra big buffers.
  - Elementwise combine ops are split DVE/GPSIMD to balance engine load.

Self-contained: hardcodes B=8, L=4096, H=N=512, 8 cores.
"""

import numpy as np
from contextlib import ExitStack

import concourse.bass as bass
import concourse.bacc as bacc
import concourse.mybir as mybir
import concourse.tile as tile

P = 128
L = 4096
H = 512
N = 512
BSZ = 8
SEG = 512                # Bu matmul / evac granularity (one PSUM bank)
NSEG = L // SEG          # 8
SLAB = 1024              # scan + untwiddle granularity
NSLAB = L // SLAB        # 4
TSLAB = 2048             # twiddle-in granularity
NTSLAB = L // TSLAB      # 2
KH = H // P              # 4 contraction chunks for Bu
NCH = 2 * N // P         # 8 real-channel chunks (re 0..3, im 4..7)
CCH = N // P             # 4 complex-channel chunks (0,1 fwd; 2,3 bwd)
NT = L // P              # 32 time blocks for the output matmul

F16 = mybir.dt.float16
F32 = mybir.dt.float32
MULT = mybir.AluOpType.mult
ADD = mybir.AluOpType.add

C_ORDER = [0, 2, 1, 3]

_CACHED = None


def _is_fwd_chunk(nch: int) -> bool:
    return (nch % 4) < 2


def build_nc():
    nc = bacc.Bacc("TRN2", target_bir_lowering=False, debug=False)
    uT = nc.declare_dram_parameter("uT", [H, L], F16, isOutput=False)
    cosT = nc.declare_dram_parameter("cosT", [N, L], F16, isOutput=False)
    sinT = nc.declare_dram_parameter("sinT", [N, L], F16, isOutput=False)
    rdec = nc.declare_dram_parameter("rdec", [P, CCH], F32, isOutput=False)
    Bcat = nc.declare_dram_parameter("Bcat", [H, 2 * N], F16, isOutput=False)
    Ccat = nc.declare_dram_parameter("Ccat", [2 * N, H], F16, isOutput=False)
    y = nc.declare_dram_parameter("y", [L, H], F32, isOutput=True)

    with tile.TileContext(nc) as tc, ExitStack() as ctx:
        const = ctx.enter_context(tc.tile_pool(name="const", bufs=1))
        big = ctx.enter_context(tc.tile_pool(name="big", bufs=1))
        upool = ctx.enter_context(tc.tile_pool(name="upool", bufs=1))
        pscr = ctx.enter_context(tc.tile_pool(name="pscr", bufs=4))
        qscr = ctx.enter_context(tc.tile_pool(name="qscr", bufs=6))
        wpool = ctx.enter_context(tc.tile_pool(name="wpool", bufs=7))
        ysb = ctx.enter_context(tc.tile_pool(name="ysb", bufs=2))
        bup = ctx.enter_context(tc.tile_pool(name="bup", bufs=6, space="PSUM"))
        yp = ctx.enter_context(tc.tile_pool(name="yp", bufs=2, space="PSUM"))

        # uT streamed in halves: cols [0:2048] then [2048:4096]
        u_t = [upool.tile([P, TSLAB], F16, tag=f"uT{k}", name=f"uT{k}")
               for k in range(KH)]
        cosb = [big.tile([P, L], F16, tag=f"cos{c}", name=f"cos{c}") for c in range(CCH)]
        sinb = [big.tile([P, L], F16, tag=f"sin{c}", name=f"sin{c}") for c in range(CCH)]
        v = [big.tile([P, L], F16, tag=f"v{j}", name=f"v{j}") for j in range(NCH)]
        bmat = [const.tile([P, 2 * N], F16, tag=f"B{k}", name=f"Bm{k}") for k in range(KH)]
        cmat = [const.tile([P, H], F16, tag=f"C{k}", name=f"Cm{k}") for k in range(NCH)]
        rdec_t = const.tile([P, CCH], F32, tag="rdec", name="rdec_t")

        # ---- constant DMAs (uT half 0 + Bcat first so Bu starts ASAP;
        #      tables next for twiddle-in; Ccat last, needed only by phase E)
        nc.sync.dma_start(rdec_t[:], rdec[:])
        for k in range(KH):
            nc.sync.dma_start(bmat[k][:], Bcat[k * P:(k + 1) * P, :])
        for k in range(KH):
            nc.sync.dma_start(u_t[k][:], uT[k * P:(k + 1) * P, 0:TSLAB])
        for c in range(CCH):
            nc.sync.dma_start(cosb[c][:], cosT[c * P:(c + 1) * P, :])
            nc.sync.dma_start(sinb[c][:], sinT[c * P:(c + 1) * P, :])
        for k in range(NCH):
            nc.sync.dma_start(cmat[k][:], Ccat[k * P:(k + 1) * P, :])

        # ---- Phase A: Bu matmuls, evacuate into v slots (scan-time order) ----
        # uhalf 0 covers tsegs {0..3} (first-needed by fwd chunks), uhalf 1
        # covers tsegs {4..7} (first-needed by bwd chunks, reversed).  All
        # half-0 groups run first (uT tiles are reloaded for half 1); within
        # each half, chunk-pair priority interleave.
        g_half0 = [0, 4, 2, 6, 1, 5, 3, 7]
        g_half1 = [2, 6, 0, 4, 3, 7, 1, 5]

        def do_group(nch, h):
            tsegs = [0, 1, 2, 3] if h == 0 else [7, 6, 5, 4]
            for ts in tsegs:
                ps = bup.tile([P, SEG], F32, name=f"bups{ts}", tag="bup")
                ucol = ts * SEG - h * TSLAB
                for k in range(KH):
                    nc.tensor.matmul(
                        ps[:],
                        bmat[k][:, nch * P:(nch + 1) * P],
                        u_t[k][:, ucol:ucol + SEG],
                        start=(k == 0), stop=(k == KH - 1),
                    )
                if _is_fwd_chunk(nch):
                    nc.scalar.copy(v[nch][:, ts * SEG:(ts + 1) * SEG], ps[:])
                else:
                    ss = NSEG - 1 - ts
                    dst = v[nch][:, ss * SEG:(ss + 1) * SEG]
                    nc.scalar.copy(dst[:, ::-1], ps[:])

        for nch in g_half0:
            do_group(nch, 0)
        for k in range(KH):
            nc.sync.dma_start(u_t[k][:], uT[k * P:(k + 1) * P, TSLAB:L])
        for nch in g_half1:
            do_group(nch, 1)

        # ---- Phases B/C/D: twiddle-in (2048), scan+untwiddle (1024) ----
        # Emitted in dependency-feasible stream order (DVE executes in order):
        #  S1: needs only uT half 0  -> fwd chunks, scan-slabs 0..1
        #  S2: needs half 1          -> bwd slabs 0..1 + fwd slabs 2..3
        #  S3: bwd slabs 2..3
        prev_w = [None] * (2 * CCH)

        def twiddle_in(c, tsl):
            jre, jim = c, c + CCH
            sl = slice(tsl * TSLAB, (tsl + 1) * TSLAB)
            cs, sn = cosb[c][:, sl], sinb[c][:, sl]
            vre, vim = v[jre][:, sl], v[jim][:, sl]
            p2 = pscr.tile([P, TSLAB], F16, tag="p", name="p2")
            p3 = pscr.tile([P, TSLAB], F16, tag="p", name="p3")
            p4 = pscr.tile([P, TSLAB], F16, tag="p", name="p4")
            nc.vector.tensor_mul(p2[:], sn, vim)
            nc.vector.tensor_mul(p4[:], sn, vre)
            nc.vector.tensor_mul(p3[:], cs, vim)
            nc.vector.tensor_mul(vre, cs, vre)   # in place: br dead after p4
            nc.gpsimd.dma_start(vre, p2[:], accum_op=mybir.AluOpType.add)
            nc.vector.tensor_sub(vim, p3[:], p4[:])

        def scan_untw(c, sb):
            jre, jim = c, c + CCH
            sl = slice(sb * SLAB, (sb + 1) * SLAB)
            cs, sn = cosb[c][:, sl], sinb[c][:, sl]
            vre, vim = v[jre][:, sl], v[jim][:, sl]
            r_ap = rdec_t[:, c:c + 1].broadcast_to((P, SLAB))
            init_r = 0.0 if sb == 0 else prev_w[jre][:, SLAB - 1:SLAB]
            init_i = 0.0 if sb == 0 else prev_w[jim][:, SLAB - 1:SLAB]
            wr = wpool.tile([P, SLAB], F16, tag="w", name="wr")
            wi = wpool.tile([P, SLAB], F16, tag="w", name="wi")
            nc.vector.tensor_tensor_scan(wr[:], r_ap, vre, init_r,
                                         op0=MULT, op1=ADD)
            nc.vector.tensor_tensor_scan(wi[:], r_ap, vim, init_i,
                                         op0=MULT, op1=ADD)
            prev_w[jre], prev_w[jim] = wr, wi

            q1 = qscr.tile([P, SLAB], F16, tag="q", name="q1")
            q2 = qscr.tile([P, SLAB], F16, tag="q", name="q2")
            q4 = qscr.tile([P, SLAB], F16, tag="q", name="q4")
            if c < 2:
                # xr = q1 - q2 (DVE); xi written as q3 into the spent v slab
                # then += q4 via SWDGE accum dma
                nc.vector.tensor_mul(q1[:], cs, wr[:])
                nc.vector.tensor_mul(q2[:], sn, wi[:])
                nc.vector.tensor_mul(vim, sn, wr[:])
                nc.vector.tensor_mul(q4[:], cs, wi[:])
                nc.vector.tensor_sub(vre, q1[:], q2[:])
                nc.gpsimd.dma_start(vim, q4[:], accum_op=mybir.AluOpType.add)
            else:
                # reversed READS flip scan order back to t order for free
                # (2x mode is kept for step -1); t-slab (NSLAB-1-sb) lands at
                # table slab sb, t-ascending
                nc.vector.tensor_mul(q1[:], cs[:, ::-1], wr[:, ::-1])
                nc.vector.tensor_mul(q2[:], sn[:, ::-1], wi[:, ::-1])
                nc.vector.tensor_mul(q4[:], sn[:, ::-1], wr[:, ::-1])
                q3r = qscr.tile([P, SLAB], F16, tag="q", name="q3r")
                nc.vector.tensor_mul(q3r[:], cs[:, ::-1], wi[:, ::-1])
                nc.vector.tensor_sub(cs, q1[:], q2[:])
                nc.vector.tensor_add(sn, q4[:], q3r[:])

        # x source for the output matmul: real chunk k, time block i.
        # bwd x: t-slab (i//8) lives at table slab (NSLAB-1-i//8), t-ordered.
        def x_src(k: int, i: int):
            if _is_fwd_chunk(k):
                return v[k][:, i * P:(i + 1) * P]
            c = k % 4
            col = (NSLAB - 1 - i // 8) * SLAB + (i % 8) * P
            src = cosb[c] if k < 4 else sinb[c]
            return src[:, col:col + P]

        def y_slab(sj):
            for o in range(SLAB // P):
                i = sj * (SLAB // P) + o
                py = yp.tile([P, H], F32, name="py", tag="yp")
                for k in range(NCH):
                    nc.tensor.matmul(
                        py[:], x_src(k, i), cmat[k][:],
                        start=(k == 0), stop=(k == NCH - 1),
                    )
                yt = ysb.tile([P, H], F32, tag="y", name="yt")
                nc.scalar.copy(yt[:], py[:])
                nc.sync.dma_start(y[i * P:(i + 1) * P, :], yt[:])

        # S1
        twiddle_in(0, 0)
        twiddle_in(1, 0)
        for sb in (0, 1):
            for c in (0, 1):
                scan_untw(c, sb)
        # S2: fwd slabs 2..3, then bwd slabs 0..1 (y slabs 3, 2 unblock)
        twiddle_in(0, 1)
        twiddle_in(1, 1)
        twiddle_in(2, 0)
        twiddle_in(3, 0)
        scan_untw(0, 2)
        scan_untw(0, 3)
        scan_untw(1, 2)
        scan_untw(1, 3)
        scan_untw(2, 0)
        scan_untw(3, 0)
        y_slab(3)
        scan_untw(2, 1)
        scan_untw(3, 1)
        y_slab(2)
        # S3: bwd slabs 2..3 (completes y slabs 1 then 0)
        twiddle_in(2, 1)
        twiddle_in(3, 1)
        scan_untw(2, 2)
        scan_untw(3, 2)
        y_slab(1)
        scan_untw(2, 3)
        scan_untw(3, 3)
        y_slab(0)

    nc.compile()
    return nc


def prepare_inputs(u, lengths, nu_log, theta_log, B, C):
    """Host-side prep: per-core in_maps. All heavy math in fp64 for accuracy."""
    u = np.asarray(u)
    lengths = np.asarray(lengths)
    nu = np.exp(np.asarray(nu_log, np.float64))
    theta = np.exp(np.asarray(theta_log, np.float64))
    r = np.exp(-nu)                                    # |lam|, (N,)

    j = np.arange(L, dtype=np.float64)
    ang = np.mod(theta[:, None] * j[None, :], 2 * np.pi)   # (N, L)
    cos_base = np.cos(ang).astype(np.float16)
    sin_base = np.sin(ang).astype(np.float16)

    Bcat = np.empty((H, 2 * N), np.float16)
    Bcat[:, :N] = np.asarray(B)[..., 0]
    Bcat[:, N:] = np.asarray(B)[..., 1]
    Ccat = np.empty((2 * N, H), np.float16)
    Ccat[:N] = np.asarray(C)[0]
    Ccat[N:] = -np.asarray(C)[1]
    rdec = r.reshape(CCH, P).T.astype(np.float32).copy()   # (128, 4)

    half = N // 2
    in_maps = []
    for b in range(BSZ):
        ln = int(lengths[b])
        ub = np.array(u[b], np.float32)
        if ln < L:
            ub[ln:, :] = 0.0
        uTh = np.ascontiguousarray(ub.T.astype(np.float16))
        cosb = cos_base.copy()
        sinb = sin_base.copy()
        if ln < L:
            cosb[:half, ln:] = 0
            sinb[:half, ln:] = 0
            cosb[half:, :L - ln] = 0
            sinb[half:, :L - ln] = 0
        in_maps.append({
            "uT": uTh, "cosT": cosb, "sinT": sinb,
            "rdec": rdec, "Bcat": Bcat, "Ccat": Ccat,
        })
    return in_maps


def kernel(u, lengths, nu_log, theta_log, B, C):
    global _CACHED
    from concourse.bass_utils import run_bass_kernel_spmd
    in_maps = prepare_inputs(u, lengths, nu_log, theta_log, B, C)
    if _CACHED is None:
        _CACHED = build_nc()
    res = run_bass_kernel_spmd(_CACHED, in_maps, list(range(BSZ)))
    y = np.stack([res.results[i]["y"] for i in range(BSZ)], axis=0)
    return y.astype(np.float32)
